# revision 8
# baseline (speedup 1.0000x reference)
"""Distributed ISTFT kernel for Trainium2 (8 NeuronCores, Bass/Tile).

Math (matches the jax reference):
  z: [2, 513, T] one-sided spectrum (real/imag), T = 8192 frames.
  Hermitian extension + ifft(1024) + window + overlap-add (hop 256) +
  divide by overlapped window sum + trim 512 each side -> [2, 2096896].

Key folds used here:
  * real(ifft) = A^T @ X where A [1024(k), 1024(n)] packs the cos rows for
    zr bins 0..512 and sin rows for zi bins 1..511; X packs those z rows.
  * imag(ifft)[n, t] = (zi[0,t] + (-1)^n zi[512,t]) / N  (rank-2).
  * Output sample m = 256*b + r; block b = sum_{q=0..3} wf_{b-q}[256q+r].
    Folding window * A, the reciprocal window-sum (a pure per-column
    scale since n = 256q + r) AND a x512 fp16-range scale into the
    stationary operand gives O^T[t, r] = sum_q X[:, t+3-q]^T @ Aw_q
    directly -- overlap-add, windowing and normalization all ride
    inside the matmul; eviction is a single 1/512 scalar multiply.
  * Everything window-derived (window sums, reciprocals, boundary-block
    fixup rows, channel-1 taps) is precomputed on the HOST, so the
    device program is a pure fp16 matmul machine: stream x + Aw chunks,
    matmul, evict.  fp16 operands halve DMA and weight-load time vs
    fp32r; PSUM accumulation stays fp32.
  * Frame axis is sharded 1024 output blocks/core with a 3-frame input
    halo, so no cross-core communication is needed at all.  The two
    blocks whose window-sum misses a frame (global block 0 and 8190)
    get a host-computed row fixup (identity rows on the other cores).
"""

import numpy as np

N_FFT = 1024
HOP = 256
T_FRAMES = 8192
N_CORES = 8
F_SLOTS = 1027  # frame slots per core: 1024 owned blocks need slots t..t+3
NB = 1024       # output blocks computed per core (core 7 uses 1023)
SCALE = 512.0   # fp16 range scale folded into aw/taps, undone at eviction

_CACHE = {}


def _amat() -> np.ndarray:
    """A [1024(kappa), 1024(n)]: ifft cos/sin weights, f64."""
    n = np.arange(N_FFT, dtype=np.float64)[None, :]
    k = np.arange(513, dtype=np.float64)[:, None]
    g = np.full((513, 1), 2.0)
    g[0, 0] = 1.0
    g[512, 0] = 1.0
    C = (g / N_FFT) * np.cos(2.0 * np.pi * k * n / N_FFT)
    k2 = np.arange(1, 512, dtype=np.float64)[:, None]
    S = (-2.0 / N_FFT) * np.sin(2.0 * np.pi * k2 * n / N_FFT)
    return np.ascontiguousarray(np.concatenate([C, S], 0))


def _build_nc():
    from contextlib import ExitStack

    import concourse.tile as tile
    from concourse import bacc, mybir

    f16 = mybir.dt.float16
    f32 = mybir.dt.float32

    nc = bacc.Bacc("TRN2", target_bir_lowering=False, debug=False,
                   num_devices=N_CORES)

    x_d = nc.dram_tensor("x", [1026, F_SLOTS], f16, kind="ExternalInput")
    a_d = nc.dram_tensor("aw", [1024, 1024], f16, kind="ExternalInput")
    t_d = nc.dram_tensor("tuv", [8, NB], f16, kind="ExternalInput")
    p_d = nc.dram_tensor("taps", [8, 256], f16, kind="ExternalInput")
    o_d = nc.dram_tensor("out", [2, NB, 256], f16, kind="ExternalOutput")

    with tile.TileContext(nc) as tc, ExitStack() as ctx:
        big = ctx.enter_context(tc.tile_pool(name="big", bufs=1))
        sml = ctx.enter_context(tc.tile_pool(name="sml", bufs=1))
        ps0p = ctx.enter_context(tc.tile_pool(name="ps0p", bufs=6, space="PSUM"))
        ps1p = ctx.enter_context(tc.tile_pool(name="ps1p", bufs=2, space="PSUM"))
        osb = ctx.enter_context(tc.tile_pool(name="osb", bufs=8))

        # small setup inputs on the gpsimd queue (off the big streams)
        tu = sml.tile([4, NB], f16, tag="tu")
        nc.gpsimd.dma_start(out=tu[:], in_=t_d.ap()[0:4, :])
        tv = sml.tile([4, NB], f16, tag="tv")
        nc.gpsimd.dma_start(out=tv[:], in_=t_d.ap()[4:8, :])
        tpu = sml.tile([4, 256], f16, tag="tpu")
        nc.gpsimd.dma_start(out=tpu[:], in_=p_d.ap()[0:4, :])
        tpv = sml.tile([4, 256], f16, tag="tpv")
        nc.gpsimd.dma_start(out=tpv[:], in_=p_d.ap()[4:8, :])

        # PE warmup: data-independent junk matmuls ramp the tensor-engine
        # clock out of its low p-state while the first real DMAs land.
        wrm = sml.tile([128, 256], f16, tag="wrm")
        nc.vector.memset(wrm[:], 0.0)
        psj = ps1p.tile([128, 256], f32, tag="ps1", name="psj")
        for i in range(10):
            nc.tensor.matmul(psj[:], lhsT=wrm[:, 0:128], rhs=wrm[:],
                             start=(i == 0), stop=(i == 9))

        # big streams: x chunks on sync queue, aw chunks on scalar queue.
        # xs0/aw0 are split so the first real matmul can start sooner.
        xs = []
        aw = []
        for k in range(8):
            xk = big.tile([128, F_SLOTS], f16, tag=f"xs{k}", name=f"xs{k}")
            if k == 0:
                nc.sync.dma_start(out=xk[:, 0:259],
                                  in_=x_d.ap()[0:128, 0:259])
                nc.sync.dma_start(out=xk[:, 259:F_SLOTS],
                                  in_=x_d.ap()[0:128, 259:F_SLOTS])
            else:
                nc.sync.dma_start(out=xk[:],
                                  in_=x_d.ap()[128 * k:128 * (k + 1), :])
            xs.append(xk)
            awk = big.tile([128, N_FFT], f16, tag=f"aw{k}", name=f"aw{k}")
            if k == 0:
                for q in range(4):
                    cols = slice(256 * q, 256 * (q + 1))
                    nc.scalar.dma_start(out=awk[:, cols],
                                        in_=a_d.ap()[0:128, cols])
            else:
                nc.scalar.dma_start(out=awk[:],
                                    in_=a_d.ap()[128 * k:128 * (k + 1), :])
            aw.append(awk)

        def evict(ps, tt, ch, queue):
            o = osb.tile([128, 256], f16, tag=f"o{ch}", name=f"o{ch}_{tt}")
            nc.vector.tensor_scalar_mul(o[:], ps[:], 1.0 / SCALE)
            queue.dma_start(
                out=o_d.ap()[ch:ch + 1, tt * 128:(tt + 1) * 128, :], in_=o[:])

        # channel 1 runs entirely up-front: its tiny inputs land first, the
        # 32 matmuls fill the PE while the big streams arrive (and keep the
        # clock ramp going); stores go on the otherwise-idle gpsimd queue.
        for tt in range(8):
            ps1 = ps1p.tile([128, 256], f32, tag="ps1", name=f"ps1_{tt}")
            nc.tensor.matmul(ps1[:], lhsT=tu[:, tt * 128:tt * 128 + 128],
                             rhs=tpu[:], start=True, stop=False)
            nc.tensor.matmul(ps1[:], lhsT=tv[:, tt * 128:tt * 128 + 128],
                             rhs=tpv[:], start=False, stop=True)
            evict(ps1, tt, 1, nc.gpsimd)

        # channel 0: k-outer accumulation in two psum sweeps; eviction
        # stores go on the scalar queue (idle once aw is in), keeping the
        # sync queue free to stream x chunks.
        def sweep(tts):
            pss = {
                tt: ps0p.tile([128, 256], f32, tag="ps0", name=f"ps0_{tt}")
                for tt in tts
            }
            for k in range(8):
                for tt in tts:
                    for q in range(4):
                        off = tt * 128 + 3 - q
                        nc.tensor.matmul(
                            pss[tt][:],
                            lhsT=xs[k][:, off:off + 128],
                            rhs=aw[k][:, 256 * q:256 * (q + 1)],
                            start=(k == 0 and q == 0),
                            stop=(k == 7 and q == 3))
            for tt in tts:
                evict(pss[tt], tt, 0, nc.scalar)

        sweep([0, 1, 2, 3, 4, 5])
        sweep([6, 7])

    nc.compile()
    return nc


def _host_consts(window: np.ndarray):
    """Window-derived constants shared by all cores (f64 math)."""
    w = window.astype(np.float64)
    w4 = w.reshape(4, 256)
    ws4 = w4.sum(axis=0)
    ws4 = np.maximum(ws4, 1e-6)
    ws3a = np.maximum(ws4 - w4[3], 1e-6)   # block 0: missing q=3 frame
    ws3b = np.maximum(ws4 - w4[0], 1e-6)   # block 8190: missing q=0 frame
    rws_tiled = 1.0 / np.tile(ws4, 4)      # per-n normalization, n=256q+r

    amat = _CACHE.get("amat")
    if amat is None:
        amat = _amat()
        _CACHE["amat"] = amat
    aw = (amat * (w * rws_tiled * SCALE)[None, :]).astype(np.float16)

    sgn = 1.0 - 2.0 * (np.arange(256) % 2)
    taps = np.empty((8, 256), np.float64)
    taps[0:4] = w4 / ws4[None, :] * (SCALE / N_FFT)
    taps[4:8] = taps[0:4] * sgn[None, :]
    taps = taps.astype(np.float16)

    r0 = (ws4 / ws3a).astype(np.float32)   # rescale for global block 0
    r7 = (ws4 / ws3b).astype(np.float32)   # rescale for global block 8190
    return aw, taps, r0, r7


def _inputs_for_cores(z: np.ndarray, window: np.ndarray):
    aw, taps, _, _ = _host_consts(window)

    in_maps = []
    for c in range(N_CORES):
        G = 1024 * c - 1  # global frame index of slot 0
        X = np.zeros((1026, F_SLOTS), np.float16)
        lo, hi = max(0, G), min(T_FRAMES, G + F_SLOTS)
        s0, s1 = lo - G, hi - G
        X[0:513, s0:s1] = z[0, :, lo:hi]
        X[513:1024, s0:s1] = z[1, 1:512, lo:hi]
        X[1024, s0:s1] = z[1, 0, lo:hi]
        X[1025, s0:s1] = z[1, 512, lo:hi]
        tuv = np.empty((8, NB), np.float16)
        for q in range(4):
            tuv[q] = X[1024, 3 - q:3 - q + NB]
            tuv[4 + q] = X[1025, 3 - q:3 - q + NB]
        in_maps.append({
            "x": X,
            "aw": aw,
            "tuv": tuv,
            "taps": taps,
        })
    return in_maps


def kernel(z: np.ndarray, window: np.ndarray) -> np.ndarray:
    from concourse.bass_utils import run_bass_kernel_spmd

    z = np.asarray(z, dtype=np.float32)
    window = np.asarray(window, dtype=np.float32)

    nc = _CACHE.get("nc")
    if nc is None:
        nc = _build_nc()
        _CACHE["nc"] = nc

    in_maps = _inputs_for_cores(z, window)
    res = run_bass_kernel_spmd(nc, in_maps, list(range(N_CORES)))

    parts = []
    for c in range(N_CORES):
        nb = NB if c < N_CORES - 1 else NB - 1
        o = res.results[c]["out"]  # [2, NB, 256] f16
        parts.append(o[:, :nb, :].reshape(2, -1))
    out = np.concatenate(parts, axis=1).astype(np.float32)
    # boundary blocks whose window-sum misses a frame: rescale on host
    _, _, r0, r7 = _host_consts(window)
    out[:, 0:256] *= r0[None, :]
    out[:, -256:] *= r7[None, :]
    return np.ascontiguousarray(out)


# revision 9
# speedup vs baseline: 1.0738x; 1.0738x over previous
"""Distributed ISTFT kernel for Trainium2 (8 NeuronCores, Bass/Tile).

Math (matches the jax reference):
  z: [2, 513, T] one-sided spectrum (real/imag), T = 8192 frames.
  Hermitian extension + ifft(1024) + window + overlap-add (hop 256) +
  divide by overlapped window sum + trim 512 each side -> [2, 2096896].

Key folds used here:
  * real(ifft) = A^T @ X where A [1024(k), 1024(n)] packs the cos rows for
    zr bins 0..512 and sin rows for zi bins 1..511; X packs those z rows.
  * imag(ifft)[n, t] = (zi[0,t] + (-1)^n zi[512,t]) / N  (rank-2).
  * Output sample m = 256*b + r; block b = sum_{q=0..3} wf_{b-q}[256q+r].
    Folding window * A, the reciprocal window-sum (a pure per-column
    scale since n = 256q + r) AND a x512 fp16-range scale into the
    stationary operand gives O^T[t, r] = sum_q X[:, t+3-q]^T @ Aw_q
    directly -- overlap-add, windowing and normalization all ride
    inside the matmul; eviction is a single 1/512 scalar multiply.
  * Everything window-derived (window sums, reciprocals, boundary-block
    fixup rows, channel-1 taps) is precomputed on the HOST, so the
    device program is a pure fp16 matmul machine: stream x + Aw chunks,
    matmul, evict.  fp16 operands halve DMA and weight-load time vs
    fp32r; PSUM accumulation stays fp32.
  * Frame axis is sharded 1024 output blocks/core with a 3-frame input
    halo, so no cross-core communication is needed at all.  The two
    blocks whose window-sum misses a frame (global block 0 and 8190)
    get a host-computed row fixup (identity rows on the other cores).
"""

import numpy as np

N_FFT = 1024
HOP = 256
T_FRAMES = 8192
N_CORES = 8
F_SLOTS = 1027  # frame slots per core: 1024 owned blocks need slots t..t+3
NB = 1024       # output blocks computed per core (core 7 uses 1023)
SCALE = 512.0   # fp16 range scale folded into aw/taps, undone at eviction

_CACHE = {}


def _amat() -> np.ndarray:
    """A [1024(kappa), 1024(n)]: ifft cos/sin weights, f64."""
    n = np.arange(N_FFT, dtype=np.float64)[None, :]
    k = np.arange(513, dtype=np.float64)[:, None]
    g = np.full((513, 1), 2.0)
    g[0, 0] = 1.0
    g[512, 0] = 1.0
    C = (g / N_FFT) * np.cos(2.0 * np.pi * k * n / N_FFT)
    k2 = np.arange(1, 512, dtype=np.float64)[:, None]
    S = (-2.0 / N_FFT) * np.sin(2.0 * np.pi * k2 * n / N_FFT)
    return np.ascontiguousarray(np.concatenate([C, S], 0))


def _build_nc():
    from contextlib import ExitStack

    import concourse.tile as tile
    from concourse import bacc, mybir

    f16 = mybir.dt.float16
    f32 = mybir.dt.float32

    nc = bacc.Bacc("TRN2", target_bir_lowering=False, debug=False,
                   num_devices=N_CORES)

    x_d = nc.dram_tensor("x", [1026, F_SLOTS], f16, kind="ExternalInput")
    a_d = nc.dram_tensor("aw", [1024, 1024], f16, kind="ExternalInput")
    t_d = nc.dram_tensor("tuv", [8, NB], f16, kind="ExternalInput")
    p_d = nc.dram_tensor("taps", [8, 256], f16, kind="ExternalInput")
    o_d = nc.dram_tensor("out", [2, NB, 256], f16, kind="ExternalOutput")

    with tile.TileContext(nc) as tc, ExitStack() as ctx:
        big = ctx.enter_context(tc.tile_pool(name="big", bufs=1))
        sml = ctx.enter_context(tc.tile_pool(name="sml", bufs=1))
        ps0p = ctx.enter_context(tc.tile_pool(name="ps0p", bufs=6, space="PSUM"))
        ps1p = ctx.enter_context(tc.tile_pool(name="ps1p", bufs=2, space="PSUM"))
        osb = ctx.enter_context(tc.tile_pool(name="osb", bufs=8))

        # small setup inputs on the gpsimd queue (off the big streams)
        tu = sml.tile([4, NB], f16, tag="tu")
        nc.gpsimd.dma_start(out=tu[:], in_=t_d.ap()[0:4, :])
        tv = sml.tile([4, NB], f16, tag="tv")
        nc.gpsimd.dma_start(out=tv[:], in_=t_d.ap()[4:8, :])
        tpu = sml.tile([4, 256], f16, tag="tpu")
        nc.gpsimd.dma_start(out=tpu[:], in_=p_d.ap()[0:4, :])
        tpv = sml.tile([4, 256], f16, tag="tpv")
        nc.gpsimd.dma_start(out=tpv[:], in_=p_d.ap()[4:8, :])

        # big streams: x chunks on sync queue, aw chunks on scalar queue.
        # xs0/aw0 are split so the first real matmul can start sooner.
        xs = []
        aw = []
        for k in range(8):
            xk = big.tile([128, F_SLOTS], f16, tag=f"xs{k}", name=f"xs{k}")
            if k == 0:
                nc.sync.dma_start(out=xk[:, 0:259],
                                  in_=x_d.ap()[0:128, 0:259])
                nc.sync.dma_start(out=xk[:, 259:F_SLOTS],
                                  in_=x_d.ap()[0:128, 259:F_SLOTS])
            else:
                nc.sync.dma_start(out=xk[:],
                                  in_=x_d.ap()[128 * k:128 * (k + 1), :])
            xs.append(xk)
            awk = big.tile([128, N_FFT], f16, tag=f"aw{k}", name=f"aw{k}")
            if k == 0:
                for q in range(4):
                    cols = slice(256 * q, 256 * (q + 1))
                    nc.scalar.dma_start(out=awk[:, cols],
                                        in_=a_d.ap()[0:128, cols])
            else:
                nc.scalar.dma_start(out=awk[:],
                                    in_=a_d.ap()[128 * k:128 * (k + 1), :])
            aw.append(awk)

        def evict(ps, tt, ch, queue):
            o = osb.tile([128, 256], f16, tag=f"o{ch}", name=f"o{ch}_{tt}")
            nc.vector.tensor_scalar_mul(o[:], ps[:], 1.0 / SCALE)
            queue.dma_start(
                out=o_d.ap()[ch:ch + 1, tt * 128:(tt + 1) * 128, :], in_=o[:])

        def ch1_group(tt):
            ps1 = ps1p.tile([128, 256], f32, tag="ps1", name=f"ps1_{tt}")
            nc.tensor.matmul(ps1[:], lhsT=tu[:, tt * 128:tt * 128 + 128],
                             rhs=tpu[:], start=True, stop=False)
            nc.tensor.matmul(ps1[:], lhsT=tv[:, tt * 128:tt * 128 + 128],
                             rhs=tpv[:], start=False, stop=True)
            evict(ps1, tt, 1, nc.gpsimd)

        # channel 0: k-outer accumulation in two psum sweeps with the tiny
        # channel-1 groups interleaved; eviction stores go on the scalar
        # (ch0) and gpsimd (ch1) queues, keeping the sync queue free to
        # stream x chunks.
        def sweep(tts, ch1_sched):
            pss = {
                tt: ps0p.tile([128, 256], f32, tag="ps0", name=f"ps0_{tt}")
                for tt in tts
            }
            for k in range(8):
                for tt in tts:
                    for q in range(4):
                        off = tt * 128 + 3 - q
                        nc.tensor.matmul(
                            pss[tt][:],
                            lhsT=xs[k][:, off:off + 128],
                            rhs=aw[k][:, 256 * q:256 * (q + 1)],
                            start=(k == 0 and q == 0),
                            stop=(k == 7 and q == 3))
                for c1 in ch1_sched.get(k, []):
                    ch1_group(c1)
            for tt in tts:
                evict(pss[tt], tt, 0, nc.scalar)

        sweep([0, 1, 2, 3, 4, 5], {k: [k - 2] for k in range(2, 8)})
        sweep([6, 7], {0: [6], 1: [7]})

    nc.compile()
    return nc


def _host_consts(window: np.ndarray):
    """Window-derived constants shared by all cores (f64 math)."""
    w = window.astype(np.float64)
    w4 = w.reshape(4, 256)
    ws4 = w4.sum(axis=0)
    ws4 = np.maximum(ws4, 1e-6)
    ws3a = np.maximum(ws4 - w4[3], 1e-6)   # block 0: missing q=3 frame
    ws3b = np.maximum(ws4 - w4[0], 1e-6)   # block 8190: missing q=0 frame
    rws_tiled = 1.0 / np.tile(ws4, 4)      # per-n normalization, n=256q+r

    amat = _CACHE.get("amat")
    if amat is None:
        amat = _amat()
        _CACHE["amat"] = amat
    aw = (amat * (w * rws_tiled * SCALE)[None, :]).astype(np.float16)

    sgn = 1.0 - 2.0 * (np.arange(256) % 2)
    taps = np.empty((8, 256), np.float64)
    taps[0:4] = w4 / ws4[None, :] * (SCALE / N_FFT)
    taps[4:8] = taps[0:4] * sgn[None, :]
    taps = taps.astype(np.float16)

    r0 = (ws4 / ws3a).astype(np.float32)   # rescale for global block 0
    r7 = (ws4 / ws3b).astype(np.float32)   # rescale for global block 8190
    return aw, taps, r0, r7


def _inputs_for_cores(z: np.ndarray, window: np.ndarray):
    aw, taps, _, _ = _host_consts(window)

    in_maps = []
    for c in range(N_CORES):
        G = 1024 * c - 1  # global frame index of slot 0
        X = np.zeros((1026, F_SLOTS), np.float16)
        lo, hi = max(0, G), min(T_FRAMES, G + F_SLOTS)
        s0, s1 = lo - G, hi - G
        X[0:513, s0:s1] = z[0, :, lo:hi]
        X[513:1024, s0:s1] = z[1, 1:512, lo:hi]
        X[1024, s0:s1] = z[1, 0, lo:hi]
        X[1025, s0:s1] = z[1, 512, lo:hi]
        tuv = np.empty((8, NB), np.float16)
        for q in range(4):
            tuv[q] = X[1024, 3 - q:3 - q + NB]
            tuv[4 + q] = X[1025, 3 - q:3 - q + NB]
        in_maps.append({
            "x": X,
            "aw": aw,
            "tuv": tuv,
            "taps": taps,
        })
    return in_maps


def kernel(z: np.ndarray, window: np.ndarray) -> np.ndarray:
    from concourse.bass_utils import run_bass_kernel_spmd

    z = np.asarray(z, dtype=np.float32)
    window = np.asarray(window, dtype=np.float32)

    nc = _CACHE.get("nc")
    if nc is None:
        nc = _build_nc()
        _CACHE["nc"] = nc

    in_maps = _inputs_for_cores(z, window)
    res = run_bass_kernel_spmd(nc, in_maps, list(range(N_CORES)))

    parts = []
    for c in range(N_CORES):
        nb = NB if c < N_CORES - 1 else NB - 1
        o = res.results[c]["out"]  # [2, NB, 256] f16
        parts.append(o[:, :nb, :].reshape(2, -1))
    out = np.concatenate(parts, axis=1).astype(np.float32)
    # boundary blocks whose window-sum misses a frame: rescale on host
    _, _, r0, r7 = _host_consts(window)
    out[:, 0:256] *= r0[None, :]
    out[:, -256:] *= r7[None, :]
    return np.ascontiguousarray(out)
